# revision 1
# baseline (speedup 1.0000x reference)
"""MoRALinear fused kernel for 8x TRN2 NeuronCores.

Math: reference computes
    y = x @ W.T + b + tile(lora_A(chunk_sum(x)))
Both the chunk-sum (x -> [B,S,r] by summing 4 chunks of 1024) and the
output tiling (repeat r->4096) are linear maps made of stacked identity
blocks, so the adapter folds into the base weight exactly:
    y = x @ (W + tile(A, (4,4))).T + b
The device kernel is then a single dense GEMM [16384,4096]x[4096,4096]
plus a bias, data-parallel over tokens across 8 cores (weights
replicated; no collectives).

Per-core kernel (M=2048 tokens, K=4096, N=4096, fp32 via float32r):
  - x block of 1024 tokens cached in SBUF ([128, 32, 1024], 16 MB)
  - W streamed as [128, 4, 512] k-chunks, reused across 8 m-strips
  - 8 PSUM banks hold the 8 m-strips of one n-tile; k accumulates in PSUM
  - bias added during PSUM->SBUF eviction on the vector engine
"""

import numpy as np

import concourse.bass as bass
import concourse.mybir as mybir
import concourse.tile as tile
from concourse import bacc
from concourse.bass_utils import run_bass_kernel_spmd

B, S, IN_F, OUT_F = 4, 4096, 4096, 4096
N_CORES = 8
TOKENS = B * S                  # 16384
M_PER_CORE = TOKENS // N_CORES  # 2048

P = 128
KO = IN_F // P                  # 32 k-strips
NTILE = 512
NT = OUT_F // NTILE             # 8 n-tiles
KO_CHUNK = 4                    # k-strips per W DMA (1 MB transfers)
MB = 1024                       # tokens per SBUF-cached x block

F32 = mybir.dt.float32
F32R = mybir.dt.float32r


def build_nc(m_per_core: int = M_PER_CORE, mb: int = MB):
    assert m_per_core % P == 0
    mb = min(mb, m_per_core)
    m_blocks = -(-m_per_core // mb)
    nc = bacc.Bacc("TRN2", target_bir_lowering=False, debug=False)

    with tile.TileContext(nc) as tc:
        with tc.tile_pool(name="dram", bufs=1, space="DRAM") as dram:
            # x swizzled block-major: row blk*KO+ko holds k-strip ko of token
            # block blk, so each chunk load is one contiguous run/partition
            n_blocks_decl = -(-m_per_core // mb)
            xt = dram.tile(
                [P, n_blocks_decl * KO, mb], F32R, kind="ExternalInput", name="xt",
                uniquify=False,
            )
            # W swizzled n-tile-major: row nt*KO+ko holds W_eff.T[k-strip ko,
            # n-slice nt] so each wk load is one contiguous 8KB run/partition
            wt = dram.tile(
                [P, NT * KO, NTILE], F32R, kind="ExternalInput", name="wt",
                uniquify=False,
            )
            bias_in = dram.tile(
                [P, OUT_F], F32, kind="ExternalInput", name="bias", uniquify=False
            )
            out_d = dram.tile(
                [P, m_per_core // P, OUT_F], F32, kind="ExternalOutput", name="out",
                uniquify=False,
            )

        n_chunks = KO // KO_CHUNK
        with (
            tc.tile_pool(name="const", bufs=1) as const,
            tc.tile_pool(name="xpool", bufs=n_chunks + 1) as xpool,
            tc.tile_pool(name="wpool", bufs=4) as wpool,
            tc.tile_pool(name="opool", bufs=4) as opool,
            tc.tile_pool(name="pspool", bufs=8, space="PSUM") as pspool,
        ):
            bias_sb = const.tile([P, OUT_F], F32, name="bias_sb")

            # xchunks[ko4] holds the resident x k-chunk for the current block.
            # A block's chunk DMAs are emitted interleaved into the weight
            # stream (block 0: n-tile 0; later blocks: previous block's last
            # n-tile) so the sync queue's ring order matches consumption
            # order and the PE never waits behind a bulk x reload.
            xchunks = [None] * n_chunks
            xnext = [None] * n_chunks

            def load_xchunk(blk, ko4, cur_mb):
                row = blk * KO + ko4 * KO_CHUNK
                xc = xpool.tile([P, KO_CHUNK, cur_mb], F32R, name="xc")
                nc.sync.dma_start(
                    xc[:], xt[:, row : row + KO_CHUNK, :cur_mb]
                )
                return xc

            for blk in range(m_blocks):
                cur_mb = min(mb, m_per_core - blk * mb)
                mt = cur_mb // P
                next_mb = min(mb, m_per_core - (blk + 1) * mb)

                for nt in range(NT):
                    ns = slice(nt * NTILE, (nt + 1) * NTILE)
                    psums = [
                        pspool.tile([P, NTILE], F32, name="ps") for _ in range(mt)
                    ]
                    for ko4 in range(n_chunks):
                        wrow = nt * KO + ko4 * KO_CHUNK
                        first_tile = blk == 0 and nt == 0
                        if first_tile:
                            # kj-granular interleaved loads: 0.75 MB arrival
                            # granularity keeps PE wait gaps below the HAM
                            # re-throttle window during the cold start
                            xchunks[ko4] = xpool.tile(
                                [P, KO_CHUNK, cur_mb], F32R, name="xc"
                            )
                            wk = wpool.tile([P, KO_CHUNK, NTILE], F32R, name="wk")
                            for kj in range(KO_CHUNK):
                                row = ko4 * KO_CHUNK + kj
                                nc.sync.dma_start(
                                    wk[:, kj : kj + 1, :],
                                    wt[:, wrow + kj : wrow + kj + 1, :],
                                )
                                nc.sync.dma_start(
                                    xchunks[ko4][:, kj : kj + 1, :],
                                    xt[:, row : row + 1, :cur_mb],
                                )
                            if ko4 == 2:
                                nc.sync.dma_start(bias_sb[:], bias_in[:])
                        else:
                            wk = wpool.tile([P, KO_CHUNK, NTILE], F32R, name="wk")
                            nc.sync.dma_start(
                                wk[:], wt[:, wrow : wrow + KO_CHUNK, :]
                            )
                            if blk >= 1 and nt == 0 and ko4 == 3:
                                # this block's last x chunk, deferred so the
                                # tile's early weight chunks lead it in ring
                                xchunks[n_chunks - 1] = load_xchunk(
                                    blk, n_chunks - 1, cur_mb
                                )
                            if blk + 1 < m_blocks and nt == NT - 1 and ko4 >= 2:
                                # next block's chunks trail one weight chunk
                                xnext[ko4 - 1] = load_xchunk(
                                    blk + 1, ko4 - 1, next_mb
                                )
                        for kj in range(KO_CHUNK):
                            ko = ko4 * KO_CHUNK + kj
                            last_k = ko == KO - 1
                            for m in range(mt):
                                nc.tensor.matmul(
                                    psums[m][:],
                                    lhsT=xchunks[ko4][
                                        :, kj : kj + 1, m * P : (m + 1) * P
                                    ],
                                    rhs=wk[:, kj : kj + 1, :],
                                    start=(ko == 0),
                                    stop=last_k,
                                )
                                if last_k:
                                    # evict as soon as this strip's
                                    # accumulation is complete
                                    ot = opool.tile([P, NTILE], F32, name="ot")
                                    nc.vector.tensor_add(
                                        out=ot[:],
                                        in0=psums[m][:],
                                        in1=bias_sb[:, ns],
                                    )
                                    nc.sync.dma_start(
                                        out_d[:, blk * (mb // P) + m, ns], ot[:]
                                    )
                        # chunk 0 of the next block loads a tile early via the
                        # spare pool slot
                        if blk + 1 < m_blocks and nt == NT - 2 and ko4 == 0:
                            xnext[0] = load_xchunk(blk + 1, 0, next_mb)
                    if nt == NT - 1 and blk + 1 < m_blocks:
                        # chunks 0..6 were prefetched; chunk 7 is deferred to
                        # the next block's first tile
                        for c in range(n_chunks - 1):
                            xchunks[c] = xnext[c]

    nc.compile()
    return nc


def prep_inputs(x, W, b, A, m_per_core: int = M_PER_CORE, n_cores: int = N_CORES):
    """Host-side shard + layout prep. Returns in_maps for run_bass_kernel_spmd."""
    x = np.asarray(x, dtype=np.float32)
    W = np.asarray(W, dtype=np.float32)
    b = np.asarray(b, dtype=np.float32)
    A = np.asarray(A, dtype=np.float32)

    r = A.shape[0]
    w_eff = W + np.tile(A, (OUT_F // r, IN_F // r))
    # wt[p, nt*KO + ko, j] = w_eff[nt*512 + j, ko*128 + p]
    wt = np.ascontiguousarray(
        w_eff.reshape(NT, NTILE, KO, P).transpose(3, 0, 2, 1).reshape(P, NT * KO, NTILE)
    )
    bias = np.ascontiguousarray(np.broadcast_to(b, (P, OUT_F)))

    x_flat = x.reshape(TOKENS, IN_F)
    n_blocks = -(-m_per_core // MB)
    in_maps = []
    for c in range(n_cores):
        shard = x_flat[c * m_per_core : (c + 1) * m_per_core]
        # xt[p, blk*KO + ko, m] = shard[blk*MB + m, ko*128 + p]
        xt = np.ascontiguousarray(
            shard.reshape(n_blocks, MB, KO, P)
            .transpose(3, 0, 2, 1)
            .reshape(P, n_blocks * KO, MB)
        )
        in_maps.append({"xt": xt, "wt": wt, "bias": bias})
    return in_maps


def unshard(results, m_per_core: int = M_PER_CORE):
    shards = []
    for res in results:
        o = res["out"]  # [P, m_per_core//P, OUT_F]; token = strip*128 + p
        shards.append(o.transpose(1, 0, 2).reshape(m_per_core, OUT_F))
    return np.concatenate(shards, axis=0).reshape(B, S, OUT_F)


_NC_CACHE = {}


def run(x, W, b, A, trace=False, tmpdir=None, **spmd_kwargs):
    key = (M_PER_CORE, MB)
    if key not in _NC_CACHE:
        _NC_CACHE[key] = build_nc()
    nc = _NC_CACHE[key]
    in_maps = prep_inputs(x, W, b, A)
    br = run_bass_kernel_spmd(
        nc, in_maps, list(range(N_CORES)), trace=trace, tmpdir=tmpdir, **spmd_kwargs
    )
    return unshard(br.results), br


def kernel(x, W, b, A):
    last_err = None
    for attempt in range(3):
        try:
            out, _ = run(x, W, b, A)
            return out.astype(np.float32)
        except Exception as e:  # transient device flakes (e.g. NRT exec errors)
            last_err = e
            _NC_CACHE.clear()
            import time

            time.sleep(5)
    raise last_err



# revision 2
# speedup vs baseline: 1.0978x; 1.0978x over previous
"""MoRALinear fused kernel for 8x TRN2 NeuronCores — bf16 v2.

Math: reference computes
    y = x @ W.T + b + tile(lora_A(chunk_sum(x)))
Both the chunk-sum (x -> [B,S,r] by summing 4 chunks of 1024) and the
output tiling (repeat r->4096) are linear maps made of stacked identity
blocks, so the adapter folds into the base weight exactly:
    y = x @ (W + tile(A, (4,4))).T + b
The device kernel is then a single dense GEMM [16384,4096]x[4096,4096]
plus a bias, data-parallel over tokens across 8 cores (weights
replicated; no collectives).

v2 vs v1: operands are bf16 (PE runs bf16 at the same 1 elem/cycle as
float32r, but LDWEIGHTS gets FWL and HBM traffic halves; the whole
2048-token x shard fits in SBUF). W is streamed exactly once; for each
n-tile its 8 k-chunks stay resident and are reused by both 1024-token
m-groups. Accumulation stays fp32 in PSUM; bias is added during
PSUM->SBUF eviction on the vector engine, which emits bf16 (2x DVE rate,
half the out-DMA bytes; host upcasts).

Per-core kernel (M=2048 tokens, K=4096, N=4096):
  - x resident in SBUF as 16 chunks [128, 4, 1024] bf16 (128 KiB/part)
  - W per n-tile: 8 chunks [128, 4, 512] bf16 (wpool ring of 10)
  - 8 PSUM banks hold the 8 m-strips of one (n-tile, m-group)
  - loop: nt (8) -> mg (2) -> ko4 (8) -> kj (4) -> m (8); the last ko4
    chunk runs m-outer so the 8 evictions spread over ~7 us of matmuls
    instead of bunching into a serial tail chain.
"""

import numpy as np
import ml_dtypes

import concourse.bass as bass
import concourse.mybir as mybir
import concourse.tile as tile
from concourse import bacc
from concourse.bass_utils import run_bass_kernel_spmd

B, S, IN_F, OUT_F = 4, 4096, 4096, 4096
N_CORES = 8
TOKENS = B * S                  # 16384
M_PER_CORE = TOKENS // N_CORES  # 2048

P = 128
KO = IN_F // P                  # 32 k-strips
NTILE = 512
NT = OUT_F // NTILE             # 8 n-tiles
KO_CHUNK = 4                    # k-strips per chunk
MB = 1024                       # tokens per m-group (8 PSUM banks x 128)

F32 = mybir.dt.float32
BF16 = mybir.dt.bfloat16
NP_BF16 = ml_dtypes.bfloat16


def build_nc(m_per_core: int = M_PER_CORE, mb: int = MB):
    assert m_per_core % mb == 0
    m_groups = m_per_core // mb          # 2
    n_chunks = KO // KO_CHUNK            # 8
    mt = mb // P                         # 8 m-strips per group
    nc = bacc.Bacc("TRN2", target_bir_lowering=False, debug=False)

    with tile.TileContext(nc) as tc:
        with tc.tile_pool(name="dram", bufs=1, space="DRAM") as dram:
            # x swizzled group-major: row mg*KO+ko holds k-strip ko of token
            # group mg, so each chunk load is one contiguous run/partition
            xt = dram.tile(
                [P, m_groups * KO, mb], BF16, kind="ExternalInput", name="xt",
                uniquify=False,
            )
            # W swizzled n-tile-major: row nt*KO+ko holds W_eff.T[k-strip ko,
            # n-slice nt] so each chunk load is one contiguous run/partition
            wt = dram.tile(
                [P, NT * KO, NTILE], BF16, kind="ExternalInput", name="wt",
                uniquify=False,
            )
            bias_in = dram.tile(
                [P, OUT_F], F32, kind="ExternalInput", name="bias", uniquify=False
            )
            out_d = dram.tile(
                [P, m_per_core // P, OUT_F], BF16, kind="ExternalOutput", name="out",
                uniquify=False,
            )

        with (
            tc.tile_pool(name="const", bufs=1) as const,
            tc.tile_pool(name="xpool", bufs=m_groups * n_chunks) as xpool,
            tc.tile_pool(name="wpool", bufs=n_chunks + 2) as wpool,
            tc.tile_pool(name="opool", bufs=8) as opool,
            tc.tile_pool(name="pspool", bufs=8, space="PSUM") as pspool,
        ):
            bias_sb = const.tile([P, OUT_F], F32, name="bias_sb")

            # x chunks stay resident for the whole kernel
            xchunks = [[None] * n_chunks for _ in range(m_groups)]
            wk_cur = [None] * n_chunks   # W chunks of the current n-tile
            wk_next = [None] * n_chunks  # prefetched chunks of the next n-tile

            def load_xchunk(mg, ko4, kj_granular=False):
                xc = xpool.tile([P, KO_CHUNK, mb], BF16, name="xc")
                row = mg * KO + ko4 * KO_CHUNK
                if kj_granular:
                    for kj in range(KO_CHUNK):
                        nc.sync.dma_start(
                            xc[:, kj : kj + 1, :], xt[:, row + kj : row + kj + 1, :]
                        )
                else:
                    nc.sync.dma_start(xc[:], xt[:, row : row + KO_CHUNK, :])
                return xc

            def load_wchunk(nt, ko4, kj_granular=False):
                wk = wpool.tile([P, KO_CHUNK, NTILE], BF16, name="wk")
                row = nt * KO + ko4 * KO_CHUNK
                if kj_granular:
                    for kj in range(KO_CHUNK):
                        nc.sync.dma_start(
                            wk[:, kj : kj + 1, :], wt[:, row + kj : row + kj + 1, :]
                        )
                else:
                    nc.sync.dma_start(wk[:], wt[:, row : row + KO_CHUNK, :])
                return wk

            def do_mm(psums, xc, wk, kj, m, ko4, nt, mg):
                ko = ko4 * KO_CHUNK + kj
                nc.tensor.matmul(
                    psums[m][:],
                    lhsT=xc[:, kj : kj + 1, m * P : (m + 1) * P],
                    rhs=wk[:, kj : kj + 1, :],
                    start=(ko == 0),
                    stop=(ko == KO - 1),
                )
                if ko == KO - 1:
                    ns = slice(nt * NTILE, (nt + 1) * NTILE)
                    ot = opool.tile([P, NTILE], BF16, name="ot")
                    nc.vector.tensor_add(
                        out=ot[:], in0=psums[m][:], in1=bias_sb[:, ns]
                    )
                    nc.sync.dma_start(out_d[:, mg * mt + m, ns], ot[:])

            for nt in range(NT):
                for mg in range(m_groups):
                    psums = [
                        pspool.tile([P, NTILE], F32, name="ps") for _ in range(mt)
                    ]
                    for ko4 in range(n_chunks):
                        if nt == 0 and mg == 0:
                            # cold start: kj-granular interleaved loads give
                            # fine arrival granularity so the PE starts early
                            wk_cur[ko4] = load_wchunk(0, ko4, kj_granular=True)
                            xchunks[0][ko4] = load_xchunk(0, ko4, kj_granular=True)
                            if ko4 == 2:
                                nc.sync.dma_start(bias_sb[:], bias_in[:])
                        wk = wk_cur[ko4]
                        xc = xchunks[mg][ko4]
                        if ko4 == n_chunks - 1:
                            # m-outer: each strip finishes its k-accumulation
                            # 4 MMs apart, spreading evictions across ~7us
                            for m in range(mt):
                                for kj in range(KO_CHUNK):
                                    do_mm(psums, xc, wk, kj, m, ko4, nt, mg)
                        else:
                            for kj in range(KO_CHUNK):
                                for m in range(mt):
                                    do_mm(psums, xc, wk, kj, m, ko4, nt, mg)
                        # prefetch, interleaved so the DMA ring order matches
                        # consumption order:
                        #  - during nt0/mg0: x chunks of mg1 trail one chunk
                        #  - during each mg1: W chunks of nt+1 stream in
                        if nt == 0 and mg == 0 and ko4 >= 1:
                            xchunks[1][ko4 - 1] = load_xchunk(1, ko4 - 1)
                        if mg == m_groups - 1 and nt + 1 < NT:
                            wk_next[ko4] = load_wchunk(nt + 1, ko4)
                    if nt == 0 and mg == 0:
                        xchunks[1][n_chunks - 1] = load_xchunk(1, n_chunks - 1)
                if nt + 1 < NT:
                    for c in range(n_chunks):
                        wk_cur[c] = wk_next[c]

    nc.compile()
    return nc


def prep_inputs(x, W, b, A, m_per_core: int = M_PER_CORE, n_cores: int = N_CORES):
    """Host-side shard + layout prep. Returns in_maps for run_bass_kernel_spmd."""
    x = np.asarray(x, dtype=np.float32)
    W = np.asarray(W, dtype=np.float32)
    b = np.asarray(b, dtype=np.float32)
    A = np.asarray(A, dtype=np.float32)

    r = A.shape[0]
    w_eff = W + np.tile(A, (OUT_F // r, IN_F // r))
    # wt[p, nt*KO + ko, j] = w_eff[nt*512 + j, ko*128 + p]
    wt = np.ascontiguousarray(
        w_eff.reshape(NT, NTILE, KO, P)
        .transpose(3, 0, 2, 1)
        .reshape(P, NT * KO, NTILE)
        .astype(NP_BF16)
    )
    bias = np.ascontiguousarray(np.broadcast_to(b, (P, OUT_F)))

    x_flat = x.reshape(TOKENS, IN_F)
    m_groups = m_per_core // MB
    in_maps = []
    for c in range(n_cores):
        shard = x_flat[c * m_per_core : (c + 1) * m_per_core]
        # xt[p, mg*KO + ko, m] = shard[mg*MB + m, ko*128 + p]
        xt = np.ascontiguousarray(
            shard.reshape(m_groups, MB, KO, P)
            .transpose(3, 0, 2, 1)
            .reshape(P, m_groups * KO, MB)
            .astype(NP_BF16)
        )
        in_maps.append({"xt": xt, "wt": wt, "bias": bias})
    return in_maps


def unshard(results, m_per_core: int = M_PER_CORE):
    shards = []
    for res in results:
        o = np.asarray(res["out"], dtype=np.float32)
        # [P, m_per_core//P, OUT_F]; token = strip*128 + p
        shards.append(o.transpose(1, 0, 2).reshape(m_per_core, OUT_F))
    return np.concatenate(shards, axis=0).reshape(B, S, OUT_F)


_NC_CACHE = {}


def run(x, W, b, A, trace=False, tmpdir=None, **spmd_kwargs):
    key = (M_PER_CORE, MB)
    if key not in _NC_CACHE:
        _NC_CACHE[key] = build_nc()
    nc = _NC_CACHE[key]
    in_maps = prep_inputs(x, W, b, A)
    br = run_bass_kernel_spmd(
        nc, in_maps, list(range(N_CORES)), trace=trace, tmpdir=tmpdir, **spmd_kwargs
    )
    return unshard(br.results), br


def kernel(x, W, b, A):
    last_err = None
    for attempt in range(3):
        try:
            out, _ = run(x, W, b, A)
            return out.astype(np.float32)
        except Exception as e:  # transient device flakes (e.g. NRT exec errors)
            last_err = e
            _NC_CACHE.clear()
            import time

            time.sleep(5)
    raise last_err


# revision 5
# speedup vs baseline: 1.1056x; 1.0071x over previous
"""MoRALinear fused kernel for 8x TRN2 NeuronCores — bf16 v2.

Math: reference computes
    y = x @ W.T + b + tile(lora_A(chunk_sum(x)))
Both the chunk-sum (x -> [B,S,r] by summing 4 chunks of 1024) and the
output tiling (repeat r->4096) are linear maps made of stacked identity
blocks, so the adapter folds into the base weight exactly:
    y = x @ (W + tile(A, (4,4))).T + b
The device kernel is then a single dense GEMM [16384,4096]x[4096,4096]
plus a bias, data-parallel over tokens across 8 cores (weights
replicated; no collectives).

v2 vs v1: operands are bf16 (PE runs bf16 at the same 1 elem/cycle as
float32r, but LDWEIGHTS gets FWL and HBM traffic halves; the whole
2048-token x shard fits in SBUF). W is streamed exactly once; for each
n-tile its 8 k-chunks stay resident and are reused by both 1024-token
m-groups. Accumulation stays fp32 in PSUM; bias is added during
PSUM->SBUF eviction on the vector engine, which emits bf16 (2x DVE rate,
half the out-DMA bytes; host upcasts).

Per-core kernel (M=2048 tokens, K=4096, N=4096):
  - x resident in SBUF as 16 chunks [128, 4, 1024] bf16 (128 KiB/part)
  - W per n-tile: 8 chunks [128, 4, 512] bf16 (wpool ring of 10)
  - 8 PSUM banks hold the 8 m-strips of one (n-tile, m-group)
  - loop: nt (8) -> mg (2) -> ko4 (8) -> kj (4) -> m (8); the last ko4
    chunk runs m-outer so the 8 evictions spread over ~7 us of matmuls
    instead of bunching into a serial tail chain.
"""

import numpy as np
import ml_dtypes

import concourse.bass as bass
import concourse.mybir as mybir
import concourse.tile as tile
from concourse import bacc
from concourse.bass_utils import run_bass_kernel_spmd

B, S, IN_F, OUT_F = 4, 4096, 4096, 4096
N_CORES = 8
TOKENS = B * S                  # 16384
M_PER_CORE = TOKENS // N_CORES  # 2048

P = 128
KO = IN_F // P                  # 32 k-strips
NTILE = 512
NT = OUT_F // NTILE             # 8 n-tiles
KO_CHUNK = 4                    # k-strips per chunk
MB = 1024                       # tokens per m-group (8 PSUM banks x 128)

F32 = mybir.dt.float32
BF16 = mybir.dt.bfloat16
NP_BF16 = ml_dtypes.bfloat16


def build_nc(m_per_core: int = M_PER_CORE, mb: int = MB):
    assert m_per_core % mb == 0
    m_groups = m_per_core // mb          # 2
    n_chunks = KO // KO_CHUNK            # 8
    mt = mb // P                         # 8 m-strips per group
    nc = bacc.Bacc("TRN2", target_bir_lowering=False, debug=False)

    with tile.TileContext(nc) as tc:
        with tc.tile_pool(name="dram", bufs=1, space="DRAM") as dram:
            # x swizzled group-major: row mg*KO+ko holds k-strip ko of token
            # group mg, so each chunk load is one contiguous run/partition
            xt = dram.tile(
                [P, m_groups * KO, mb], BF16, kind="ExternalInput", name="xt",
                uniquify=False,
            )
            # W swizzled n-tile-major: row nt*KO+ko holds W_eff.T[k-strip ko,
            # n-slice nt] so each chunk load is one contiguous run/partition
            wt = dram.tile(
                [P, NT * KO, NTILE], BF16, kind="ExternalInput", name="wt",
                uniquify=False,
            )
            bias_in = dram.tile(
                [P, OUT_F], F32, kind="ExternalInput", name="bias", uniquify=False
            )
            out_d = dram.tile(
                [P, m_per_core // P, OUT_F], BF16, kind="ExternalOutput", name="out",
                uniquify=False,
            )

        with (
            tc.tile_pool(name="const", bufs=1) as const,
            tc.tile_pool(name="xpool", bufs=m_groups * n_chunks) as xpool,
            tc.tile_pool(name="wpool", bufs=n_chunks + 2) as wpool,
            tc.tile_pool(name="opool", bufs=8) as opool,
            tc.tile_pool(name="pspool", bufs=8, space="PSUM") as pspool,
        ):
            bias_sb = const.tile([P, OUT_F], F32, name="bias_sb")

            # x chunks stay resident for the whole kernel
            xchunks = [[None] * n_chunks for _ in range(m_groups)]
            wk_cur = [None] * n_chunks   # W chunks of the current n-tile
            wk_next = [None] * n_chunks  # prefetched chunks of the next n-tile

            def load_xchunk(mg, ko4):
                xc = xpool.tile([P, KO_CHUNK, mb], BF16, name="xc")
                row = mg * KO + ko4 * KO_CHUNK
                nc.sync.dma_start(xc[:], xt[:, row : row + KO_CHUNK, :])
                return xc

            def load_cold_chunks(ko4, halve_x):
                """Cold-start loads for (nt0, mg0): x and W interleaved per
                k-strip (x leading) so the ring arrival order matches the
                kj-inner consumption order; optionally split x into token
                halves so the first m-strips' matmuls start even earlier."""
                xc = xpool.tile([P, KO_CHUNK, mb], BF16, name="xc")
                wk = wpool.tile([P, KO_CHUNK, NTILE], BF16, name="wk")
                xrow = ko4 * KO_CHUNK
                wrow = ko4 * KO_CHUNK
                for kj in range(KO_CHUNK):
                    if halve_x:
                        for h in range(2):
                            hs = slice(h * (mb // 2), (h + 1) * (mb // 2))
                            nc.sync.dma_start(
                                xc[:, kj : kj + 1, hs],
                                xt[:, xrow + kj : xrow + kj + 1, hs],
                            )
                    else:
                        nc.sync.dma_start(
                            xc[:, kj : kj + 1, :], xt[:, xrow + kj : xrow + kj + 1, :]
                        )
                    nc.sync.dma_start(
                        wk[:, kj : kj + 1, :], wt[:, wrow + kj : wrow + kj + 1, :]
                    )
                return xc, wk

            def load_wchunk(nt, ko4):
                wk = wpool.tile([P, KO_CHUNK, NTILE], BF16, name="wk")
                row = nt * KO + ko4 * KO_CHUNK
                nc.sync.dma_start(wk[:], wt[:, row : row + KO_CHUNK, :])
                return wk

            def do_mm(psums, xc, wk, kj, m, ko4, nt, mg):
                ko = ko4 * KO_CHUNK + kj
                nc.tensor.matmul(
                    psums[m][:],
                    lhsT=xc[:, kj : kj + 1, m * P : (m + 1) * P],
                    rhs=wk[:, kj : kj + 1, :],
                    start=(ko == 0),
                    stop=(ko == KO - 1),
                )
                if ko == KO - 1:
                    ns = slice(nt * NTILE, (nt + 1) * NTILE)
                    ot = opool.tile([P, NTILE], BF16, name="ot")
                    nc.vector.tensor_add(
                        out=ot[:], in0=psums[m][:], in1=bias_sb[:, ns]
                    )
                    nc.sync.dma_start(out_d[:, mg * mt + m, ns], ot[:])

            for nt in range(NT):
                for mg in range(m_groups):
                    psums = [
                        pspool.tile([P, NTILE], F32, name="ps") for _ in range(mt)
                    ]
                    for ko4 in range(n_chunks):
                        if nt == 0 and mg == 0:
                            # cold start: kj-granular interleaved loads give
                            # fine arrival granularity so the PE starts early
                            xchunks[0][ko4], wk_cur[ko4] = load_cold_chunks(
                                ko4, halve_x=(ko4 == 0)
                            )
                        wk = wk_cur[ko4]
                        xc = xchunks[mg][ko4]
                        if ko4 == n_chunks - 1:
                            # m-outer: each strip finishes its k-accumulation
                            # 4 MMs apart, spreading evictions across ~7us
                            for m in range(mt):
                                for kj in range(KO_CHUNK):
                                    do_mm(psums, xc, wk, kj, m, ko4, nt, mg)
                        else:
                            for kj in range(KO_CHUNK):
                                for m in range(mt):
                                    do_mm(psums, xc, wk, kj, m, ko4, nt, mg)
                        # prefetch, interleaved so the DMA ring order matches
                        # consumption order.  nt0/mg0 already streams 12 MB
                        # on the critical path, so it only carries bias plus
                        # the first two mg1 x chunks; the rest of mg1's x
                        # rides two steps ahead of consumption inside mg1.
                        if nt == 0 and mg == 0:
                            if ko4 == 5:
                                nc.sync.dma_start(bias_sb[:], bias_in[:])
                                xchunks[1][0] = load_xchunk(1, 0)
                            elif ko4 == 7:
                                xchunks[1][1] = load_xchunk(1, 1)
                        if nt == 0 and mg == 1 and ko4 + 2 < n_chunks:
                            xchunks[1][ko4 + 2] = load_xchunk(1, ko4 + 2)
                        if mg == m_groups - 1 and nt + 1 < NT:
                            wk_next[ko4] = load_wchunk(nt + 1, ko4)
                if nt + 1 < NT:
                    for c in range(n_chunks):
                        wk_cur[c] = wk_next[c]

    nc.compile()
    return nc


def prep_inputs(x, W, b, A, m_per_core: int = M_PER_CORE, n_cores: int = N_CORES):
    """Host-side shard + layout prep. Returns in_maps for run_bass_kernel_spmd."""
    x = np.asarray(x, dtype=np.float32)
    W = np.asarray(W, dtype=np.float32)
    b = np.asarray(b, dtype=np.float32)
    A = np.asarray(A, dtype=np.float32)

    r = A.shape[0]
    w_eff = W + np.tile(A, (OUT_F // r, IN_F // r))
    # wt[p, nt*KO + ko, j] = w_eff[nt*512 + j, ko*128 + p]
    wt = np.ascontiguousarray(
        w_eff.reshape(NT, NTILE, KO, P)
        .transpose(3, 0, 2, 1)
        .reshape(P, NT * KO, NTILE)
        .astype(NP_BF16)
    )
    bias = np.ascontiguousarray(np.broadcast_to(b, (P, OUT_F)))

    x_flat = x.reshape(TOKENS, IN_F)
    m_groups = m_per_core // MB
    in_maps = []
    for c in range(n_cores):
        shard = x_flat[c * m_per_core : (c + 1) * m_per_core]
        # xt[p, mg*KO + ko, m] = shard[mg*MB + m, ko*128 + p]
        xt = np.ascontiguousarray(
            shard.reshape(m_groups, MB, KO, P)
            .transpose(3, 0, 2, 1)
            .reshape(P, m_groups * KO, MB)
            .astype(NP_BF16)
        )
        in_maps.append({"xt": xt, "wt": wt, "bias": bias})
    return in_maps


def unshard(results, m_per_core: int = M_PER_CORE):
    shards = []
    for res in results:
        o = np.asarray(res["out"], dtype=np.float32)
        # [P, m_per_core//P, OUT_F]; token = strip*128 + p
        shards.append(o.transpose(1, 0, 2).reshape(m_per_core, OUT_F))
    return np.concatenate(shards, axis=0).reshape(B, S, OUT_F)


_NC_CACHE = {}


def run(x, W, b, A, trace=False, tmpdir=None, **spmd_kwargs):
    key = (M_PER_CORE, MB)
    if key not in _NC_CACHE:
        _NC_CACHE[key] = build_nc()
    nc = _NC_CACHE[key]
    in_maps = prep_inputs(x, W, b, A)
    br = run_bass_kernel_spmd(
        nc, in_maps, list(range(N_CORES)), trace=trace, tmpdir=tmpdir, **spmd_kwargs
    )
    return unshard(br.results), br


def kernel(x, W, b, A):
    last_err = None
    for attempt in range(3):
        try:
            out, _ = run(x, W, b, A)
            return out.astype(np.float32)
        except Exception as e:  # transient device flakes (e.g. NRT exec errors)
            last_err = e
            _NC_CACHE.clear()
            import time

            time.sleep(5)
    raise last_err
